# revision 1
# baseline (speedup 1.0000x reference)
"""DenseCRFLoss Trainium2 kernel (8-core SPMD).

Math: loss = -(WEIGHT/n) * sum_img sum_{p,q} W[p,q] * sum_k S[k,p] S[k,q]
with W = exp(-0.5*||f_p - f_q||^2), f = [xy/50, rgb/15], P = 64*64 = 4096
(inputs are first downsampled 128->64; nearest for images, 2x2-avg for segs).

Device decomposition (per core; 2 cores per image, split by row parity):
  * -0.5*d2 for a [128p x 512q] tile is ONE bf16 matmul: augmented features
    a = [f, -0.5|f|^2, 1], b = [f, 1, -0.5|f|^2], each split hi/lo into two
    bf16 vectors (28 contraction rows) so the PSUM fp32 dot is fp32-accurate.
  * W = exp(.) on ScalarE (PSUM -> SBUF, bf16 out), batched over 6-tile
    [128,1536] PSUM groups (2 groups x 3 banks double-buffered; ScalarE is
    the bottleneck engine and runs gapless in steady state).
  * T_J[k,q] += S_chunk^T @ W_tile on PE (contraction over the 128 p rows,
    accumulated in one PSUM bank across all row-chunks of column J); these
    reductions are deprioritized so PE prefers refilling the exp pipeline.
  * per column J: slab[:, J] = T_J * S_J elementwise (DVE); host sums slab.
  * symmetry: only supertiles (I <= J) at 256x256 granularity are computed
    (kept fraction 136/256 vs 144/256 at 512 granularity); off-diagonal ones
    use 2*S (pre-doubled bf16 weights) to count both sides. 256 granularity
    stays SPMD-uniform because each parity core owns exactly one 128-row
    chunk per 256-supertile, so both cores run identical instruction streams.
"""

import numpy as np
import ml_dtypes

WEIGHT = 1e-7
SIGMA_RGB = 15.0
SIGMA_XY_EFF = 50.0  # SIGMA_XY * SCALE
N, K, H, W_IN = 4, 4, 128, 128
HS = H // 2
P = HS * HS          # 4096 pixels after downsample
NCHUNK = P // 128    # 32 row chunks of 128 pixels
NJ = P // 512        # 8 column blocks of 512
ROWS_PER_CORE = NCHUNK // 2
KAUG = 28            # 7 augmented dims x (hi,lo) x cross terms
N_CORES = 8

bf16 = ml_dtypes.bfloat16

_COMPILED = None  # (nc,) cache so repeated kernel() calls reuse the module


def _split_multi_waits(nc, mybir, max_waits=1):
    """This walrus build rejects >1 sync wait per instruction. Move extra
    waits onto NoOps inserted just before the instruction (same engine, same
    bb position => engine program order preserved; waiting earlier on the
    same engine is semantically identical)."""
    for f in nc.m.functions:
        for bb in f.blocks:
            new = []
            changed = False
            for inst in bb.instructions:
                si = inst.sync_info
                if si is not None and si.on_wait and len(si.on_wait) > max_waits:
                    changed = True
                    waits = list(si.on_wait)
                    extra, keep = waits[:-max_waits], waits[-max_waits:]
                    for i in range(0, len(extra), max_waits):
                        nop = mybir.InstNoOp(
                            name=nc.get_next_instruction_name(),
                            sync_info=mybir.SyncInfo(
                                on_wait=extra[i : i + max_waits], on_update=[]
                            ),
                            bass_nofuse=True,
                            engine=inst.engine,
                        )
                        new.append(nop)
                    inst.sync_info = mybir.SyncInfo(
                        on_wait=keep, on_update=list(si.on_update or [])
                    )
                new.append(inst)
            if changed:
                bb.instructions = new


def _build_module():
    import concourse.bass as bass
    import concourse.mybir as mybir
    import concourse.tile as tile
    from contextlib import ExitStack

    f32 = mybir.dt.float32
    b16 = mybir.dt.bfloat16

    nc = bass.Bass()
    lhs_d = nc.dram_tensor("lhs", [KAUG, ROWS_PER_CORE * 128], b16, kind="ExternalInput")
    rhs_d = nc.dram_tensor("rhs", [KAUG, P], b16, kind="ExternalInput")
    sw_d = nc.dram_tensor("sw", [128, 2 * ROWS_PER_CORE * K], b16, kind="ExternalInput")
    sep_d = nc.dram_tensor("sep", [K, P], f32, kind="ExternalInput")
    acc_d = nc.dram_tensor("acc", [K, P], f32, kind="ExternalOutput")

    with tile.TileContext(nc) as tc:
        with ExitStack() as ctx:
            consts = ctx.enter_context(tc.tile_pool(name="consts", bufs=1))
            wpool = ctx.enter_context(tc.tile_pool(name="wpool", bufs=6))
            outp = ctx.enter_context(tc.tile_pool(name="outp", bufs=1))
            gpool = ctx.enter_context(
                tc.tile_pool(name="gpool", bufs=2, space="PSUM")
            )
            tpool = ctx.enter_context(
                tc.tile_pool(name="tpool", bufs=2, space="PSUM")
            )

            lhs = consts.tile([KAUG, ROWS_PER_CORE * 128], b16)
            rhs = consts.tile([KAUG, P], b16)
            sw = consts.tile([128, 2 * ROWS_PER_CORE * K], b16)
            sep = consts.tile([K, P], f32)
            slab = outp.tile([K, P], f32)
            nc.sync.dma_start(out=lhs[:], in_=lhs_d[:])
            nc.gpsimd.dma_start(out=rhs[:], in_=rhs_d[:])
            nc.scalar.dma_start(out=sw[:], in_=sw_d[:])
            nc.scalar.dma_start(out=sep[:], in_=sep_d[:])

            GROUP = 6
            QW = 256          # q-column width; triangle at 256x256 supertiles
            NJ2 = P // QW     # 16 columns
            # flat tile list, J descending: (lr, dbl, J, first/last of column).
            # At 256 granularity each core owns exactly one 128-row chunk per
            # supertile (local row lr == supertile index), so both parity
            # cores run the identical instruction stream. The I==J supertile
            # is the diagonal -> weight 1 (dbl=0); others use pre-doubled S.
            tiles_flat = []
            for J in range(NJ2 - 1, -1, -1):
                col = [(I, 1 if I != J else 0) for I in range(J + 1)]
                n = len(col)
                for t, (lr, dbl) in enumerate(col):
                    tiles_flat.append((lr, dbl, J, t == 0, t == n - 1))

            t_cur = None
            bounds = [0, 1] + list(range(1 + GROUP, len(tiles_flat), GROUP)) + [len(tiles_flat)]
            for bi in range(len(bounds) - 1):
                base = bounds[bi]
                grp = tiles_flat[base : bounds[bi + 1]]
                g = gpool.tile([128, QW * len(grp)], f32, tag="g")
                for t, (lr, dbl, J, first, last) in enumerate(grp):
                    nc.tensor.matmul(
                        g[:, QW * t : QW * (t + 1)],
                        lhs[:, 128 * lr : 128 * (lr + 1)],
                        rhs[:, QW * J : QW * (J + 1)],
                        start=True,
                        stop=True,
                    )
                w = wpool.tile([128, QW * len(grp)], b16, tag="w")
                nc.scalar.activation(
                    w[:], g[:], mybir.ActivationFunctionType.Exp
                )
                with tc.high_priority(offset=-20):
                    # deprioritize the S^T@W reduction: it is off the ACT
                    # critical path, so PE should prefer refilling G slots
                    for t, (lr, dbl, J, first, last) in enumerate(grp):
                        # each (odd J, even J-1) column pair shares one
                        # [K,512] T bank: the odd column's start=True clears
                        # the bank; the even column's first matmul
                        # (start=False) overwrites its half via the
                        # per-element has_written bits, so both accumulate
                        # correctly and one DVE epilogue covers both
                        if first and J % 2 == 1:
                            t_cur = tpool.tile([K, 512], f32, tag="t_cur")
                        off = QW if J % 2 == 1 else 0
                        nc.tensor.matmul(
                            t_cur[:, off : off + QW],
                            sw[:, (2 * lr + dbl) * K : (2 * lr + dbl + 1) * K],
                            w[:, QW * t : QW * (t + 1)],
                            start=first and J % 2 == 1,
                            stop=last,
                            skip_group_check=True,
                        )
                        if last and J % 2 == 0:
                            nc.vector.tensor_tensor(
                                slab[:, QW * J : QW * J + 512],
                                t_cur[:, 0:512],
                                sep[:, QW * J : QW * J + 512],
                                mybir.AluOpType.mult,
                            )
                            if J == 2:
                                # columns run J descending: slices 2.. are
                                # final here; ship them early so only the last
                                # two slices ride the kernel tail
                                nc.sync.dma_start(
                                    out=acc_d[:, 2 * QW :], in_=slab[:, 2 * QW :]
                                )

            nc.sync.dma_start(out=acc_d[:, 0 : 2 * QW], in_=slab[:, 0 : 2 * QW])

    _split_multi_waits(nc, mybir)
    return nc


def _prepare_core_inputs(images, segmentations):
    """Host-side prep: downsample, build augmented bf16 features, shard."""
    images = np.asarray(images, dtype=np.float32)
    segs = np.asarray(segmentations, dtype=np.float32)

    yy, xx = np.meshgrid(
        np.arange(HS, dtype=np.float32), np.arange(HS, dtype=np.float32),
        indexing="ij",
    )
    pos = np.stack([xx, yy], axis=-1).reshape(P, 2) / np.float32(SIGMA_XY_EFF)

    in_maps = []
    for m in range(N):
        img_s = images[m][:, ::2, ::2]                                  # [3,64,64]
        seg_s = segs[m].reshape(K, HS, 2, HS, 2).mean(axis=(2, 4))      # [4,64,64]
        seg_s = seg_s.astype(np.float32)
        rgb = (img_s.reshape(3, P).T / np.float32(SIGMA_RGB)).astype(np.float32)
        f = np.concatenate([pos, rgb], axis=1).astype(np.float32)       # [P,5]
        sq = (f * f).sum(axis=1).astype(np.float32)
        ones = np.ones((P, 1), np.float32)
        a7 = np.concatenate([f, -0.5 * sq[:, None], ones], 1).astype(np.float32)
        b7 = np.concatenate([f, ones, -0.5 * sq[:, None]], 1).astype(np.float32)
        hi_a = a7.astype(bf16)
        lo_a = (a7 - hi_a.astype(np.float32)).astype(bf16)
        hi_b = b7.astype(bf16)
        lo_b = (b7 - hi_b.astype(np.float32)).astype(bf16)
        a28 = np.concatenate([hi_a, hi_a, lo_a, lo_a], 1)               # [P,28] bf16
        b28 = np.concatenate([hi_b, lo_b, hi_b, lo_b], 1)               # [P,28] bf16

        S = seg_s.reshape(K, P).astype(np.float32)                      # [4,P]
        ST1 = S.T.astype(bf16)                                          # [P,4]
        ST2 = (2.0 * S.T).astype(bf16)                                  # [P,4]

        rhs = np.ascontiguousarray(b28.T)                               # [28,P]
        for par in range(2):
            rows = [i for i in range(NCHUNK) if i % 2 == par]
            lhs = np.empty((KAUG, ROWS_PER_CORE * 128), bf16)
            sw = np.empty((128, 2 * ROWS_PER_CORE * K), bf16)
            for lr, i in enumerate(rows):
                blk = slice(128 * i, 128 * (i + 1))
                lhs[:, 128 * lr : 128 * (lr + 1)] = a28[blk].T
                sw[:, (2 * lr) * K : (2 * lr + 1) * K] = ST1[blk]
                sw[:, (2 * lr + 1) * K : (2 * lr + 2) * K] = ST2[blk]
            in_maps.append({
                "lhs": lhs,
                "rhs": rhs,
                "sw": sw,
                "sep": np.ascontiguousarray(S),
            })
    return in_maps


def kernel(images, segmentations):
    from concourse.bass_utils import run_bass_kernel_spmd

    global _COMPILED
    if _COMPILED is None:
        _COMPILED = _build_module()
    nc = _COMPILED

    in_maps = _prepare_core_inputs(images, segmentations)
    res = run_bass_kernel_spmd(nc, in_maps, list(range(N_CORES)))
    total = 0.0
    for c in range(N_CORES):
        total += res.results[c]["acc"].astype(np.float64).sum()
    loss = np.float32(-WEIGHT / N) * np.float32(total)
    return np.array([loss], dtype=np.float32)



# revision 2
# speedup vs baseline: 1.3236x; 1.3236x over previous
"""DenseCRFLoss Trainium2 kernel v2 (8-core SPMD).

Math: loss = -(WEIGHT/n) * sum_img sum_{p,q} W[p,q] * sum_k S[k,p] S[k,q]
with W = exp(-0.5*||f_p - f_q||^2), f = [xy/50, rgb/15], P = 64*64 = 4096.

v2 design (per core; 2 cores per image, split by supertile-row parity):
  * PE emits y = A*(-0.5 d2) + B (A = 128/ln2, B = 16256 - c) for a
    [128p x 256q] supertile in ONE bf16 matmul: augmented features scaled by
    A, hi/lo split (28 rows) + 2 constant-B rows = 30 contraction rows.
  * exp is split across TWO engines per PSUM group [128, 256*len]:
      - ACT columns [0:cA]: exact exp via activation(scale=1/A, bias=-B/A),
        bf16 out.
      - DVE columns [cA:]: Schraudolph bit-trick: tensor_scalar(max, 0.0)
        f32->int16 (round-half-even) writes the bf16 BIT PATTERN of
        2^((y-16256)/128) ~ exp(x); per-element error +-3% with ~zero mean
        (c=7.6 tuned offline; measured total rel err ~5e-4).
    Diagonal supertiles are placed at group position 0 (always inside the
    ACT share) so W[p,p]=1 stays exact.
  * T^T-matmuls are FLIPPED: out tT[128q, 4k] += W_slice^T @ S_chunk with
    free size 4 (vs 256 in v1) -> PE cost for the reduction drops ~30x and
    the whole per-core T^T lives in ONE PSUM bank quarter [128, 128] f32,
    accumulated via the has_written-bit trick (one start=True clears the
    bank; every later matmul uses start=False).
  * epilogue: slab[128,128] = tT * sepT on DVE once at the end; host sums.
  * symmetry: supertiles I <= J at 256 granularity; off-diagonals use
    pre-doubled 2S weights to count both triangles.
"""

import numpy as np
import ml_dtypes

WEIGHT = 1e-7
SIGMA_RGB = 15.0
SIGMA_XY_EFF = 50.0  # SIGMA_XY * SCALE
N, K, H, W_IN = 4, 3, 128, 128  # W_IN unused; K of segs is 4 (set below)
KSEG = 4
HS = H // 2
P = HS * HS            # 4096 pixels after downsample
NST = P // 256         # 16 supertiles per side
KAUG = 30              # 7 aug dims x (hi,lo) x cross terms + 2 const rows
N_CORES = 8

LN2 = float(np.log(2.0))
A_SCALE = 128.0 / LN2
C_OFF = 7.6
B_OFF = 128.0 * 127.0 - C_OFF

# Per-consumer PSUM tiles (a PSUM tile may only have ONE reader engine, the
# tile framework serializes multiple readers): ACT tiles of 4 supertiles
# ([128,1024] f32 = 2 banks, bufs=2) + DVE tiles of 2 supertiles ([128,512]
# = 1 bank, bufs=3) + tT (1 bank) = 8 banks.
ACT_SIZES = [1, 2, 3] + [4] * 18   # 78 supertiles
DVE_SIZES = [1, 1] + [2] * 28      # 58 supertiles

bf16 = ml_dtypes.bfloat16

_COMPILED = None


def _split_multi_waits(nc, mybir, max_waits=1):
    """This walrus build rejects >1 sync wait per instruction. Move extra
    waits onto NoOps inserted just before the instruction (same engine, same
    bb position => engine program order preserved)."""
    for f in nc.m.functions:
        for bb in f.blocks:
            new = []
            changed = False
            for inst in bb.instructions:
                si = inst.sync_info
                if si is not None and si.on_wait and len(si.on_wait) > max_waits:
                    changed = True
                    waits = list(si.on_wait)
                    extra, keep = waits[:-max_waits], waits[-max_waits:]
                    for i in range(0, len(extra), max_waits):
                        nop = mybir.InstNoOp(
                            name=nc.get_next_instruction_name(),
                            sync_info=mybir.SyncInfo(
                                on_wait=extra[i : i + max_waits], on_update=[]
                            ),
                            bass_nofuse=True,
                            engine=inst.engine,
                        )
                        new.append(nop)
                    inst.sync_info = mybir.SyncInfo(
                        on_wait=keep, on_update=list(si.on_update or [])
                    )
                new.append(inst)
            if changed:
                bb.instructions = new


def _make_schedule():
    """Interleaved per-engine PSUM tile schedule.

    Returns a list of (engine, [(lr, dbl, J), ...]) where engine is "act" or
    "dve". Diagonal supertiles (dbl == 0) are always placed in ACT tiles so
    W[p,p] = 1 stays exact. Order is free: tT accumulation commutes."""
    diag = [(J, 0, J) for J in range(NST)]                    # I == J
    off = [(I, 1, J) for J in range(NST) for I in range(J)]   # I < J
    assert sum(ACT_SIZES) + sum(DVE_SIZES) == len(diag) + len(off) == 136

    act_tiles = []
    di, oi = 0, 0
    for s in ACT_SIZES:
        g = []
        if di < len(diag):
            g.append(diag[di])
            di += 1
            s -= 1
        g.extend(off[oi : oi + s])
        oi += s
        act_tiles.append(g)
    assert di == len(diag)
    dve_tiles = []
    for s in DVE_SIZES:
        dve_tiles.append(off[oi : oi + s])
        oi += s
    assert oi == len(off)

    # greedy interleave by scheduled engine-busy time (engine-busy ns)
    def act_cost(n):  # per ACT tile of n supertiles
        return 256 * n / 1.2 + 185
    def dve_cost(n):
        return 256 * n / 0.96 + 125

    sched = []
    ta = td = 0.0
    ai = vi = 0
    while ai < len(act_tiles) or vi < len(dve_tiles):
        if vi >= len(dve_tiles) or (ai < len(act_tiles) and ta <= td):
            sched.append(("act", act_tiles[ai]))
            ta += act_cost(len(act_tiles[ai]))
            ai += 1
        else:
            sched.append(("dve", dve_tiles[vi]))
            td += dve_cost(len(dve_tiles[vi]))
            vi += 1
    return sched


def _build_module():
    import concourse.bass as bass
    import concourse.mybir as mybir
    import concourse.tile as tile
    from contextlib import ExitStack

    f32 = mybir.dt.float32
    b16 = mybir.dt.bfloat16
    i16 = mybir.dt.int16

    nc = bass.Bass()
    # lhs: per-chunk A-side features [30, 128] x 16 chunks, split in 2 tiles
    lhs_d = [
        nc.dram_tensor(f"lhs{i}", [KAUG, 1024], b16, kind="ExternalInput")
        for i in range(2)
    ]
    # rhs: B-side features for all P columns, split in 4 tiles of 1024 cols
    rhs_d = [
        nc.dram_tensor(f"rhs{i}", [KAUG, 1024], b16, kind="ExternalInput")
        for i in range(4)
    ]
    # sw: S-chunk weights [128, (2*lr+dbl)*4 + k] (dbl=1 slots pre-doubled)
    sw_d = nc.dram_tensor("sw", [128, 2 * NST * KSEG], b16, kind="ExternalInput")
    # sepT: [p, 4*qb + k] = S[k, 128*qb + p]
    sep_d = nc.dram_tensor("sep", [128, (P // 128) * KSEG], f32, kind="ExternalInput")
    acc_d = nc.dram_tensor("acc", [128, (P // 128) * KSEG], f32, kind="ExternalOutput")

    groups = _make_schedule()

    with tile.TileContext(nc) as tc:
        with ExitStack() as ctx:
            consts = ctx.enter_context(tc.tile_pool(name="consts", bufs=1))
            wpool = ctx.enter_context(tc.tile_pool(name="wpool", bufs=3))
            wdpool = ctx.enter_context(tc.tile_pool(name="wdpool", bufs=4))
            outp = ctx.enter_context(tc.tile_pool(name="outp", bufs=1))
            gapool = ctx.enter_context(
                tc.tile_pool(name="gapool", bufs=2, space="PSUM")
            )
            gdpool = ctx.enter_context(
                tc.tile_pool(name="gdpool", bufs=3, space="PSUM")
            )
            tpool = ctx.enter_context(
                tc.tile_pool(name="tpool", bufs=1, space="PSUM")
            )

            lhs = [consts.tile([KAUG, 1024], b16, name=f"lhs{i}") for i in range(2)]
            rhs = [consts.tile([KAUG, 1024], b16, name=f"rhs{i}") for i in range(4)]
            sw = consts.tile([128, 2 * NST * KSEG], b16)
            sep = consts.tile([128, (P // 128) * KSEG], f32)
            slab = outp.tile([128, (P // 128) * KSEG], f32)
            nc.sync.dma_start(out=rhs[0][:], in_=rhs_d[0][:])
            nc.scalar.dma_start(out=lhs[0][:], in_=lhs_d[0][:])
            nc.gpsimd.dma_start(out=sw[:], in_=sw_d[:])
            nc.sync.dma_start(out=rhs[1][:], in_=rhs_d[1][:])
            nc.scalar.dma_start(out=rhs[2][:], in_=rhs_d[2][:])
            nc.sync.dma_start(out=rhs[3][:], in_=rhs_d[3][:])
            nc.scalar.dma_start(out=lhs[1][:], in_=lhs_d[1][:])
            nc.gpsimd.dma_start(out=sep[:], in_=sep_d[:])

            tT = tpool.tile([128, (P // 128) * KSEG], f32)

            bias_ap = consts.tile([128, 1], f32)
            nc.gpsimd.memset(bias_ap[:], float(-B_OFF / A_SCALE))

            first_t = True
            n_tiles_total = sum(len(g) for _, g in groups)
            n_done = 0
            for eng, g in groups:
                width = 256 * len(g)
                if eng == "act":
                    gt = gapool.tile([128, width], f32, tag="ga")
                else:
                    gt = gdpool.tile([128, width], f32, tag="gd")
                for t, (lr, dbl, J) in enumerate(g):
                    qc = J // 4          # rhs tile index
                    qo = 256 * (J % 4)   # rhs column offset
                    lc = lr // 8         # lhs tile index
                    lo = 128 * (lr % 8)
                    nc.tensor.matmul(
                        gt[:, 256 * t : 256 * (t + 1)],
                        lhs[lc][:, lo : lo + 128],
                        rhs[qc][:, qo : qo + 256],
                        start=True,
                        stop=True,
                    )
                if eng == "act":
                    w = wpool.tile([128, width], b16, tag="wact")
                    nc.scalar.activation(
                        w[:],
                        gt[:],
                        mybir.ActivationFunctionType.Exp,
                        bias=bias_ap[:],
                        scale=float(1.0 / A_SCALE),
                    )
                else:
                    w = wdpool.tile([128, width], b16, tag="wdve")
                    nc.vector.tensor_scalar(
                        out=w[:].bitcast(i16),
                        in0=gt[:],
                        scalar1=0.0,
                        scalar2=None,
                        op0=mybir.AluOpType.max,
                    )
                with tc.high_priority(offset=-20):
                    for t, (lr, dbl, J) in enumerate(g):
                        n_done += 1
                        for b in (0, 1):
                            off = 256 * t + 128 * b
                            nc.tensor.matmul(
                                tT[:, 4 * (2 * J + b) : 4 * (2 * J + b) + 4],
                                w[:, off : off + 128],
                                sw[:, (2 * lr + dbl) * KSEG : (2 * lr + dbl + 1) * KSEG],
                                start=first_t,
                                stop=(n_done == n_tiles_total and b == 1),
                                skip_group_check=True,
                            )
                            first_t = False

            nc.vector.tensor_tensor(
                slab[:], tT[:], sep[:], mybir.AluOpType.mult
            )
            nc.sync.dma_start(out=acc_d[:], in_=slab[:])

    _split_multi_waits(nc, mybir)
    return nc


def _prepare_core_inputs(images, segmentations):
    """Host-side prep: downsample, build scaled augmented bf16 features,
    shard 2 cores per image by supertile-row parity."""
    images = np.asarray(images, dtype=np.float32)
    segs = np.asarray(segmentations, dtype=np.float32)
    n = images.shape[0]

    yy, xx = np.meshgrid(
        np.arange(HS, dtype=np.float64), np.arange(HS, dtype=np.float64),
        indexing="ij",
    )
    pos = np.stack([xx, yy], axis=-1).reshape(P, 2) / float(SIGMA_XY_EFF)

    in_maps = []
    for m in range(n):
        img_s = images[m][:, ::2, ::2].astype(np.float64)               # [3,64,64]
        seg_s = segs[m].reshape(KSEG, HS, 2, HS, 2).mean(axis=(2, 4))   # [4,64,64]
        rgb = img_s.reshape(3, P).T / float(SIGMA_RGB)
        f = np.concatenate([pos, rgb], axis=1)                          # [P,5] f64
        sq = (f * f).sum(axis=1)
        a7 = np.concatenate(
            [A_SCALE * f, A_SCALE * (-0.5) * sq[:, None],
             A_SCALE * np.ones((P, 1))], 1)
        b7 = np.concatenate([f, np.ones((P, 1)), (-0.5) * sq[:, None]], 1)
        hi_a = a7.astype(bf16)
        lo_a = (a7 - hi_a.astype(np.float64)).astype(bf16)
        hi_b = b7.astype(bf16)
        lo_b = (b7 - hi_b.astype(np.float64)).astype(bf16)
        l28 = np.concatenate([hi_a, hi_a, lo_a, lo_a], 1)               # [P,28] bf16
        r28 = np.concatenate([hi_b, lo_b, hi_b, lo_b], 1)               # [P,28] bf16
        B_hi = bf16(B_OFF)
        B_lo = bf16(B_OFF - np.float64(B_hi))
        ones = np.ones((P, 1), bf16)
        l30 = np.concatenate([l28, B_hi * ones, B_lo * ones], 1)        # [P,30]
        r30 = np.concatenate([r28, ones, ones], 1)                      # [P,30]

        S = seg_s.reshape(KSEG, P).astype(np.float32)                   # [4,P]
        ST1 = S.T.astype(bf16)                                          # [P,4]
        ST2 = (2.0 * S.T.astype(np.float64)).astype(bf16)               # [P,4]

        # rhs tiles [30, 1024] per q-chunk
        rhsT = np.ascontiguousarray(r30.T.astype(bf16))                 # [30,P]
        rhs_tiles = [
            np.ascontiguousarray(rhsT[:, 1024 * c : 1024 * (c + 1)])
            for c in range(4)
        ]

        # sepT [128, 4*qb + k] = S[k, 128*qb + p]
        sep = np.empty((128, (P // 128) * KSEG), np.float32)
        for qb in range(P // 128):
            sep[:, 4 * qb : 4 * qb + 4] = S[:, 128 * qb : 128 * (qb + 1)].T

        lhsT = np.ascontiguousarray(l30.T.astype(bf16))                 # [30,P]
        for par in range(2):
            # core owns global chunk 2*I + par for supertile-row I
            lhs_tiles = [np.empty((KAUG, 1024), bf16) for _ in range(2)]
            sw = np.empty((128, 2 * NST * KSEG), bf16)
            for lr in range(NST):
                gchunk = 2 * lr + par
                blk = slice(128 * gchunk, 128 * (gchunk + 1))
                lc, lo = lr // 8, 128 * (lr % 8)
                lhs_tiles[lc][:, lo : lo + 128] = lhsT[:, blk]
                sw[:, (2 * lr) * KSEG : (2 * lr + 1) * KSEG] = ST1[blk]
                sw[:, (2 * lr + 1) * KSEG : (2 * lr + 2) * KSEG] = ST2[blk]
            im = {
                "sw": sw,
                "sep": sep,
            }
            for i in range(2):
                im[f"lhs{i}"] = lhs_tiles[i]
            for i in range(4):
                im[f"rhs{i}"] = rhs_tiles[i]
            in_maps.append(im)
    return in_maps


def kernel(images, segmentations):
    from concourse.bass_utils import run_bass_kernel_spmd

    global _COMPILED
    if _COMPILED is None:
        _COMPILED = _build_module()
    nc = _COMPILED

    in_maps = _prepare_core_inputs(images, segmentations)
    res = run_bass_kernel_spmd(nc, in_maps, list(range(N_CORES)))
    total = 0.0
    n = np.asarray(images).shape[0]
    for c in range(N_CORES):
        total += res.results[c]["acc"].astype(np.float64).sum()
    loss = np.float32(-WEIGHT / n) * np.float32(total)
    return np.array([loss], dtype=np.float32)


# revision 3
# speedup vs baseline: 1.3528x; 1.0220x over previous
"""DenseCRFLoss Trainium2 kernel v2 (8-core SPMD).

Math: loss = -(WEIGHT/n) * sum_img sum_{p,q} W[p,q] * sum_k S[k,p] S[k,q]
with W = exp(-0.5*||f_p - f_q||^2), f = [xy/50, rgb/15], P = 64*64 = 4096.

v2 design (per core; 2 cores per image, split by supertile-row parity):
  * PE emits y = A*(-0.5 d2) + B (A = 128/ln2, B = 16256 - c) for a
    [128p x 256q] supertile in ONE bf16 matmul: augmented features scaled by
    A, hi/lo split (28 rows) + 2 constant-B rows = 30 contraction rows.
  * exp is split across TWO engines per PSUM group [128, 256*len]:
      - ACT columns [0:cA]: exact exp via activation(scale=1/A, bias=-B/A),
        bf16 out.
      - DVE columns [cA:]: Schraudolph bit-trick: tensor_scalar(max, 0.0)
        f32->int16 (round-half-even) writes the bf16 BIT PATTERN of
        2^((y-16256)/128) ~ exp(x); per-element error +-3% with ~zero mean
        (c=7.6 tuned offline; measured total rel err ~5e-4).
    Diagonal supertiles are placed at group position 0 (always inside the
    ACT share) so W[p,p]=1 stays exact.
  * T^T-matmuls are FLIPPED: out tT[128q, 4k] += W_slice^T @ S_chunk with
    free size 4 (vs 256 in v1) -> PE cost for the reduction drops ~30x and
    the whole per-core T^T lives in ONE PSUM bank quarter [128, 128] f32,
    accumulated via the has_written-bit trick (one start=True clears the
    bank; every later matmul uses start=False).
  * epilogue: slab[128,128] = tT * sepT on DVE once at the end; host sums.
  * symmetry: supertiles I <= J at 256 granularity; off-diagonals use
    pre-doubled 2S weights to count both triangles.
"""

import numpy as np
import ml_dtypes

WEIGHT = 1e-7
SIGMA_RGB = 15.0
SIGMA_XY_EFF = 50.0  # SIGMA_XY * SCALE
N, K, H, W_IN = 4, 3, 128, 128  # W_IN unused; K of segs is 4 (set below)
KSEG = 4
HS = H // 2
P = HS * HS            # 4096 pixels after downsample
NST = P // 256         # 16 supertiles per side
KAUG = 30              # 7 aug dims x (hi,lo) x cross terms + 2 const rows
N_CORES = 8

LN2 = float(np.log(2.0))
A_SCALE = 128.0 / LN2
C_OFF = 7.6
B_OFF = 128.0 * 127.0 - C_OFF

# Per-consumer PSUM tiles (a PSUM tile may only have ONE reader engine, the
# tile framework serializes multiple readers): ACT tiles of 4 supertiles
# ([128,1024] f32 = 2 banks, bufs=2) + DVE tiles of 2 supertiles ([128,512]
# = 1 bank, bufs=3) + tT (1 bank) = 8 banks.
ACT_SIZES = [1, 2, 3] + [4] * 17 + [2]   # 76 supertiles
DVE_SIZES = [1, 1] + [2] * 29      # 60 supertiles

bf16 = ml_dtypes.bfloat16

_COMPILED = None


def _split_multi_waits(nc, mybir, max_waits=1):
    """This walrus build rejects >1 sync wait per instruction. Move extra
    waits onto NoOps inserted just before the instruction (same engine, same
    bb position => engine program order preserved)."""
    for f in nc.m.functions:
        for bb in f.blocks:
            new = []
            changed = False
            for inst in bb.instructions:
                si = inst.sync_info
                if si is not None and si.on_wait and len(si.on_wait) > max_waits:
                    changed = True
                    waits = list(si.on_wait)
                    extra, keep = waits[:-max_waits], waits[-max_waits:]
                    for i in range(0, len(extra), max_waits):
                        nop = mybir.InstNoOp(
                            name=nc.get_next_instruction_name(),
                            sync_info=mybir.SyncInfo(
                                on_wait=extra[i : i + max_waits], on_update=[]
                            ),
                            bass_nofuse=True,
                            engine=inst.engine,
                        )
                        new.append(nop)
                    inst.sync_info = mybir.SyncInfo(
                        on_wait=keep, on_update=list(si.on_update or [])
                    )
                new.append(inst)
            if changed:
                bb.instructions = new


def _make_schedule():
    """Interleaved per-engine PSUM tile schedule.

    Returns a list of (engine, [(lr, dbl, J), ...]) where engine is "act" or
    "dve". Diagonal supertiles (dbl == 0) are always placed in ACT tiles so
    W[p,p] = 1 stays exact. Order is free: tT accumulation commutes."""
    diag = [(J, 0, J) for J in range(NST)]                    # I == J
    off = [(I, 1, J) for J in range(NST) for I in range(J)]   # I < J
    assert sum(ACT_SIZES) + sum(DVE_SIZES) == len(diag) + len(off) == 136

    act_tiles = []
    di, oi = 0, 0
    for s in ACT_SIZES:
        g = []
        if di < len(diag):
            g.append(diag[di])
            di += 1
            s -= 1
        g.extend(off[oi : oi + s])
        oi += s
        act_tiles.append(g)
    assert di == len(diag)
    dve_tiles = []
    for s in DVE_SIZES:
        dve_tiles.append(off[oi : oi + s])
        oi += s
    assert oi == len(off)

    # greedy interleave by scheduled engine-busy time (engine-busy ns)
    def act_cost(n):  # per ACT tile of n supertiles
        return 256 * n / 1.2 + 185
    def dve_cost(n):
        return 256 * n / 0.96 + 125

    sched = []
    ta = td = 0.0
    ai = vi = 0
    while ai < len(act_tiles) or vi < len(dve_tiles):
        if vi >= len(dve_tiles) or (ai < len(act_tiles) and ta <= td):
            sched.append(("act", act_tiles[ai]))
            ta += act_cost(len(act_tiles[ai]))
            ai += 1
        else:
            sched.append(("dve", dve_tiles[vi]))
            td += dve_cost(len(dve_tiles[vi]))
            vi += 1
    return sched


def _build_module():
    import concourse.bass as bass
    import concourse.mybir as mybir
    import concourse.tile as tile
    from contextlib import ExitStack

    f32 = mybir.dt.float32
    b16 = mybir.dt.bfloat16
    i16 = mybir.dt.int16

    nc = bass.Bass()
    # lhs: per-chunk A-side features [30, 128] x 16 chunks, split in 2 tiles
    lhs_d = [
        nc.dram_tensor(f"lhs{i}", [KAUG, 1024], b16, kind="ExternalInput")
        for i in range(2)
    ]
    # rhs: B-side features for all P columns, split in 4 tiles of 1024 cols
    rhs_d = [
        nc.dram_tensor(f"rhs{i}", [KAUG, 1024], b16, kind="ExternalInput")
        for i in range(4)
    ]
    # sw: S-chunk weights [128, (2*lr+dbl)*4 + k] (dbl=1 slots pre-doubled)
    sw_d = nc.dram_tensor("sw", [128, 2 * NST * KSEG], b16, kind="ExternalInput")
    # sepT: [p, 4*qb + k] = S[k, 128*qb + p]
    sep_d = nc.dram_tensor("sep", [128, (P // 128) * KSEG], f32, kind="ExternalInput")
    acc_d = nc.dram_tensor("acc", [128, (P // 128) * KSEG], f32, kind="ExternalOutput")

    groups = _make_schedule()

    with tile.TileContext(nc) as tc:
        with ExitStack() as ctx:
            consts = ctx.enter_context(tc.tile_pool(name="consts", bufs=1))
            wpool = ctx.enter_context(tc.tile_pool(name="wpool", bufs=3))
            wdpool = ctx.enter_context(tc.tile_pool(name="wdpool", bufs=4))
            outp = ctx.enter_context(tc.tile_pool(name="outp", bufs=1))
            gapool = ctx.enter_context(
                tc.tile_pool(name="gapool", bufs=2, space="PSUM")
            )
            gdpool = ctx.enter_context(
                tc.tile_pool(name="gdpool", bufs=3, space="PSUM")
            )
            tpool = ctx.enter_context(
                tc.tile_pool(name="tpool", bufs=1, space="PSUM")
            )

            lhs = [consts.tile([KAUG, 1024], b16, name=f"lhs{i}") for i in range(2)]
            rhs = [consts.tile([KAUG, 1024], b16, name=f"rhs{i}") for i in range(4)]
            sw = consts.tile([128, 2 * NST * KSEG], b16)
            sep = consts.tile([128, (P // 128) * KSEG], f32)
            slab = outp.tile([128, (P // 128) * KSEG], f32)
            nc.sync.dma_start(out=rhs[0][:], in_=rhs_d[0][:])
            nc.scalar.dma_start(out=lhs[0][:], in_=lhs_d[0][:])
            nc.gpsimd.dma_start(out=sw[:], in_=sw_d[:])
            nc.sync.dma_start(out=rhs[1][:], in_=rhs_d[1][:])
            nc.scalar.dma_start(out=rhs[2][:], in_=rhs_d[2][:])
            nc.sync.dma_start(out=rhs[3][:], in_=rhs_d[3][:])
            nc.scalar.dma_start(out=lhs[1][:], in_=lhs_d[1][:])
            nc.gpsimd.dma_start(out=sep[:], in_=sep_d[:])

            tT = tpool.tile([128, (P // 128) * KSEG], f32)

            bias_ap = consts.tile([128, 1], f32)
            nc.gpsimd.memset(bias_ap[:], float(-B_OFF / A_SCALE))

            first_t = True
            n_tiles_total = sum(len(g) for _, g in groups)
            n_done = 0
            n_groups = len(groups)
            for g_idx, (eng, g) in enumerate(groups):
                width = 256 * len(g)
                if eng == "act":
                    gt = gapool.tile([128, width], f32, tag="ga")
                else:
                    gt = gdpool.tile([128, width], f32, tag="gd")
                for t, (lr, dbl, J) in enumerate(g):
                    qc = J // 4          # rhs tile index
                    qo = 256 * (J % 4)   # rhs column offset
                    lc = lr // 8         # lhs tile index
                    lo = 128 * (lr % 8)
                    nc.tensor.matmul(
                        gt[:, 256 * t : 256 * (t + 1)],
                        lhs[lc][:, lo : lo + 128],
                        rhs[qc][:, qo : qo + 256],
                        start=True,
                        stop=True,
                    )
                if eng == "act":
                    w = wpool.tile([128, width], b16, tag="wact")
                    nc.scalar.activation(
                        w[:],
                        gt[:],
                        mybir.ActivationFunctionType.Exp,
                        bias=bias_ap[:],
                        scale=float(1.0 / A_SCALE),
                    )
                else:
                    w = wdpool.tile([128, width], b16, tag="wdve")
                    nc.vector.tensor_scalar(
                        out=w[:].bitcast(i16),
                        in0=gt[:],
                        scalar1=0.0,
                        scalar2=None,
                        op0=mybir.AluOpType.max,
                    )
                prio = -20 if g_idx < n_groups - 4 else 0
                with tc.high_priority(offset=prio):
                    for t, (lr, dbl, J) in enumerate(g):
                        n_done += 1
                        for b in (0, 1):
                            off = 256 * t + 128 * b
                            nc.tensor.matmul(
                                tT[:, 4 * (2 * J + b) : 4 * (2 * J + b) + 4],
                                w[:, off : off + 128],
                                sw[:, (2 * lr + dbl) * KSEG : (2 * lr + dbl + 1) * KSEG],
                                start=first_t,
                                stop=(n_done == n_tiles_total and b == 1),
                                skip_group_check=True,
                            )
                            first_t = False

            nc.vector.tensor_tensor(
                slab[:], tT[:], sep[:], mybir.AluOpType.mult
            )
            nc.sync.dma_start(out=acc_d[:], in_=slab[:])

    _split_multi_waits(nc, mybir)
    return nc


def _prepare_core_inputs(images, segmentations):
    """Host-side prep: downsample, build scaled augmented bf16 features,
    shard 2 cores per image by supertile-row parity."""
    images = np.asarray(images, dtype=np.float32)
    segs = np.asarray(segmentations, dtype=np.float32)
    n = images.shape[0]

    yy, xx = np.meshgrid(
        np.arange(HS, dtype=np.float64), np.arange(HS, dtype=np.float64),
        indexing="ij",
    )
    pos = np.stack([xx, yy], axis=-1).reshape(P, 2) / float(SIGMA_XY_EFF)

    in_maps = []
    for m in range(n):
        img_s = images[m][:, ::2, ::2].astype(np.float64)               # [3,64,64]
        seg_s = segs[m].reshape(KSEG, HS, 2, HS, 2).mean(axis=(2, 4))   # [4,64,64]
        rgb = img_s.reshape(3, P).T / float(SIGMA_RGB)
        f = np.concatenate([pos, rgb], axis=1)                          # [P,5] f64
        sq = (f * f).sum(axis=1)
        a7 = np.concatenate(
            [A_SCALE * f, A_SCALE * (-0.5) * sq[:, None],
             A_SCALE * np.ones((P, 1))], 1)
        b7 = np.concatenate([f, np.ones((P, 1)), (-0.5) * sq[:, None]], 1)
        hi_a = a7.astype(bf16)
        lo_a = (a7 - hi_a.astype(np.float64)).astype(bf16)
        hi_b = b7.astype(bf16)
        lo_b = (b7 - hi_b.astype(np.float64)).astype(bf16)
        l28 = np.concatenate([hi_a, hi_a, lo_a, lo_a], 1)               # [P,28] bf16
        r28 = np.concatenate([hi_b, lo_b, hi_b, lo_b], 1)               # [P,28] bf16
        B_hi = bf16(B_OFF)
        B_lo = bf16(B_OFF - np.float64(B_hi))
        ones = np.ones((P, 1), bf16)
        l30 = np.concatenate([l28, B_hi * ones, B_lo * ones], 1)        # [P,30]
        r30 = np.concatenate([r28, ones, ones], 1)                      # [P,30]

        S = seg_s.reshape(KSEG, P).astype(np.float32)                   # [4,P]
        ST1 = S.T.astype(bf16)                                          # [P,4]
        ST2 = (2.0 * S.T.astype(np.float64)).astype(bf16)               # [P,4]

        # rhs tiles [30, 1024] per q-chunk
        rhsT = np.ascontiguousarray(r30.T.astype(bf16))                 # [30,P]
        rhs_tiles = [
            np.ascontiguousarray(rhsT[:, 1024 * c : 1024 * (c + 1)])
            for c in range(4)
        ]

        # sepT [128, 4*qb + k] = S[k, 128*qb + p]
        sep = np.empty((128, (P // 128) * KSEG), np.float32)
        for qb in range(P // 128):
            sep[:, 4 * qb : 4 * qb + 4] = S[:, 128 * qb : 128 * (qb + 1)].T

        lhsT = np.ascontiguousarray(l30.T.astype(bf16))                 # [30,P]
        for par in range(2):
            # core owns global chunk 2*I + par for supertile-row I
            lhs_tiles = [np.empty((KAUG, 1024), bf16) for _ in range(2)]
            sw = np.empty((128, 2 * NST * KSEG), bf16)
            for lr in range(NST):
                gchunk = 2 * lr + par
                blk = slice(128 * gchunk, 128 * (gchunk + 1))
                lc, lo = lr // 8, 128 * (lr % 8)
                lhs_tiles[lc][:, lo : lo + 128] = lhsT[:, blk]
                sw[:, (2 * lr) * KSEG : (2 * lr + 1) * KSEG] = ST1[blk]
                sw[:, (2 * lr + 1) * KSEG : (2 * lr + 2) * KSEG] = ST2[blk]
            im = {
                "sw": sw,
                "sep": sep,
            }
            for i in range(2):
                im[f"lhs{i}"] = lhs_tiles[i]
            for i in range(4):
                im[f"rhs{i}"] = rhs_tiles[i]
            in_maps.append(im)
    return in_maps


def kernel(images, segmentations):
    from concourse.bass_utils import run_bass_kernel_spmd

    global _COMPILED
    if _COMPILED is None:
        _COMPILED = _build_module()
    nc = _COMPILED

    in_maps = _prepare_core_inputs(images, segmentations)
    res = run_bass_kernel_spmd(nc, in_maps, list(range(N_CORES)))
    total = 0.0
    n = np.asarray(images).shape[0]
    for c in range(N_CORES):
        total += res.results[c]["acc"].astype(np.float64).sum()
    loss = np.float32(-WEIGHT / n) * np.float32(total)
    return np.array([loss], dtype=np.float32)
